# revision 22
# baseline (speedup 1.0000x reference)
"""Causal self-attention (B=4, T=2048, C=1024, H=16) on 8 TRN2 NeuronCores.

Sharding: core c handles batch element c//2 with heads (c%2)*8 .. +8
(tensor-parallel over heads, paired per batch element).

Single merged SPMD pipeline per 512-token chunk ch:
  1. QKV projection for chunk ch: Q^T,K^T accumulated as [dh,t] pairs in
     PSUM, copied to SBUF bf16 on the Pool engine; V in [t,dh] layout with
     an appended ones-column (rowsum rides the AV matmul for free).
  2. Attention for query block qb=ch (needs K/V only for chunks <= ch):
     scores S^T[k,q] = K-tile^T . Q^T restricted to the exact 128-granular
     causal triangle, computed in PSUM *pairs* so one ACT exp instruction
     covers [128, 2, 512] (amortizes ACT per-instruction overhead), causal
     edge handled by a [128,128] tril mask multiply on DVE, then AV with
     the ones-augmented V accumulates [y^T; rowsum] in PSUM.  Rowsum
     reciprocal via the fast DVE approximation, partition-broadcast on the
     Pool engine, normalize on DVE.
  3. Pairwise AllToAll exchanges token-halves of y^T (each core of a pair
     keeps 256 of each 512 query block with all 1024 features), then the
     out-projection runs only on this core's half of the tokens.
     Out-projection for block qb is emitted during attention qb+1 so the
     collective latency hides under compute.

Host side only shards/relays inputs and re-interleaves the 8 output shards.
"""

import math
import sys

import numpy as np

for _p in ("/opt/trn_rl_repo",):
    if _p not in sys.path:
        sys.path.insert(0, _p)

import ml_dtypes  # noqa: E402

import concourse.bass as bass  # noqa: E402
import concourse.bacc as bacc  # noqa: E402
import concourse.mybir as mybir  # noqa: E402
import concourse.tile as tile  # noqa: E402

FP32 = mybir.dt.float32
BF16 = mybir.dt.bfloat16
Act = mybir.ActivationFunctionType

B, T, C, H, DH = 4, 2048, 1024, 16, 64
NCORES = 8
HPC = 8        # heads per core
HPT = HPC // 2  # head-pair tiles of 128 partitions
QB = 512       # query block / token chunk
NQB = T // QB


def build_program(skip_bias=False):
    nc = bacc.Bacc(None, num_devices=NCORES)
    xT = nc.declare_dram_parameter("xT", [C, T], BF16, isOutput=False)
    wqkv = nc.declare_dram_parameter("wqkv", [C, 3 * 512], BF16, isOutput=False)
    # per-core 512-column slice of W_out (out-projection is column-split)
    wout = nc.declare_dram_parameter("wout", [C, 512], BF16, isOutput=False)
    bqk = nc.declare_dram_parameter("bqk", [128, 8], FP32, isOutput=False)
    bv = nc.declare_dram_parameter("bv", [1, 512], BF16, isOutput=False)
    bfull = nc.declare_dram_parameter("bfull", [1, 512], BF16, isOutput=False)
    maskp = nc.declare_dram_parameter("mask", [128, 128], BF16, isOutput=False)
    onesbp = nc.declare_dram_parameter("onesb", [1, 128], BF16, isOutput=False)
    out = nc.declare_dram_parameter("out", [T, 512], FP32, isOutput=True)

    groups = [[0, 1], [2, 3], [4, 5], [6, 7]]

    with tile.TileContext(nc, num_cores=NCORES) as tc:
        with (
            tc.tile_pool(name="const", bufs=1) as cpool,
            tc.tile_pool(name="dram", bufs=1, space="DRAM") as dpool,
        ):
            # ---- constants ----
            wout_sb = cpool.tile([128, 8, 512], BF16)
            for a in range(8):
                nc.sync.dma_start(out=wout_sb[:, a, :], in_=wout[a * 128:(a + 1) * 128, :])
            mask_sb = cpool.tile([128, 128], BF16)
            nc.sync.dma_start(out=mask_sb, in_=maskp[:, :])
            onesb_sb = cpool.tile([1, 128], BF16)
            nc.sync.dma_start(out=onesb_sb, in_=onesbp[:, :])
            bqk_sb = cpool.tile([128, 8], FP32)
            nc.sync.dma_start(out=bqk_sb, in_=bqk[:, :])
            bv_sb = cpool.tile([1, 512], BF16)
            nc.sync.dma_start(out=bv_sb, in_=bv[:, :])
            bfull_sb = cpool.tile([1, 512], BF16)
            nc.sync.dma_start(out=bfull_sb, in_=bfull[:, :])

            # exchange buffers (dram): per query block, local y^T half and
            # the pairwise-allgathered full y^T
            yloc = [
                dpool.tile([512, QB], BF16, tag=f"yloc{i}", name=f"yloc{i}")
                for i in range(NQB)
            ]
            yfull = [
                dpool.tile([1024, QB], BF16, tag=f"yfull{i}", name=f"yfull{i}")
                for i in range(NQB)
            ]

            with tc.tile_pool(name="persist", bufs=1) as ppool:
                # ---- persistent activations ----
                # qk=0 -> Q^T, qk=1 -> K^T, laid out [dh, qk, hp, t]
                qkT_sb = ppool.tile([128, 2, HPT, T], BF16, name="qkT")
                v_sb = ppool.tile([128, HPC, T // 128, 65], BF16, name="v")
                nc.vector.memset(v_sb[:, :, :, 64], 1.0)

                with (
                    tc.tile_pool(name="wq", bufs=1) as wpool,
                    tc.tile_pool(name="xch", bufs=2) as xpool,
                    tc.tile_pool(name="pair", bufs=2, space="PSUM") as pairp,
                    tc.tile_pool(name="accp", bufs=4, space="PSUM") as accp,
                    tc.tile_pool(name="pexp", bufs=16) as pxp,
                    tc.tile_pool(name="rr", bufs=2) as rrp,
                    tc.tile_pool(name="yun", bufs=4) as yup,
                    tc.tile_pool(name="rb", bufs=2) as rbp,
                    tc.tile_pool(name="ytc", bufs=2) as ytp,
                    tc.tile_pool(name="yf", bufs=2) as yfp,
                    tc.tile_pool(name="ob", bufs=2) as obp,
                ):
                    w_sb = wpool.tile([128, 8, 3 * 512], BF16)
                    for a in range(8):
                        nc.sync.dma_start(out=w_sb[:, a, :], in_=wqkv[a * 128:(a + 1) * 128, :])

                    def qkv_chunk(ch):
                        """QKV projection for one 512-token chunk (dense PE
                        block; PSUM->SBUF copies ride the ACT engine so the
                        DVE queue stays reserved for the normalize chain)."""
                        ts = ch * QB
                        xt = xpool.tile([128, 8, QB], BF16, tag="xt")
                        for a in range(8):
                            nc.sync.dma_start(out=xt[:, a, :], in_=xT[a * 128:(a + 1) * 128, ts:ts + QB])
                        for hp in range(HPT):
                            # Q^T and K^T pair: stationary weights, moving x^T
                            pt = pairp.tile([128, 2, QB], FP32, tag="sp")
                            for kind in range(2):  # 0=q, 1=k
                                wcol = kind * 512 + hp * 128
                                for a in range(8):
                                    nc.tensor.matmul(
                                        pt[:, kind, :],
                                        lhsT=w_sb[:, a, wcol:wcol + 128],
                                        rhs=xt[:, a, :],
                                        start=(a == 0), stop=(a == 7),
                                    )
                            dst = qkT_sb[:, :, hp, ts:ts + QB]
                            if skip_bias:
                                nc.scalar.activation(dst, pt, Act.Copy)
                            else:
                                for kind in range(2):
                                    nc.scalar.activation(
                                        dst[:, kind, :], pt[:, kind, :],
                                        Act.Identity,
                                        bias=bqk_sb[:, kind * 4 + hp:kind * 4 + hp + 1],
                                    )
                            # V in [t, d] layout: stationary x^T, moving W_v
                            accv = accp.tile([128, 512], FP32, tag="acc")
                            for a in range(8):
                                nc.tensor.matmul(
                                    accv,
                                    lhsT=xt[:, a, hp * 128:(hp + 1) * 128],
                                    rhs=w_sb[:, a, 1024:1536],
                                    start=(a == 0), stop=(skip_bias and a == 7),
                                )
                            if not skip_bias:
                                nc.tensor.matmul(
                                    accv,
                                    lhsT=onesb_sb[0:1, :],
                                    rhs=bv_sb[0:1, :],
                                    start=False, stop=True,
                                )
                            tt = ch * 4 + hp
                            nc.scalar.activation(
                                v_sb[:, :, tt, 0:64],
                                accv.rearrange("p (h d) -> p h d", d=64),
                                Act.Copy,
                            )

                    def attn_scores(h, qb):
                        """Scores + exp + causal mask for one head; returns
                        the P-tile pieces the (later) AV stage consumes."""
                        hp, off = h // 2, (h % 2) * 64
                        qs = qb * QB
                        qT = qkT_sb[off:off + 64, 0, hp, :]
                        kT = qkT_sb[off:off + 64, 1, hp, :]
                        pieces = []
                        # off-diagonal key tiles, two per PSUM pair
                        for j in range(2 * qb):
                            sp = pairp.tile([128, 2, QB], FP32, tag="sp")
                            for u in range(2):
                                kt = 2 * j + u
                                nc.tensor.matmul(
                                    sp[:, u, :],
                                    lhsT=kT[:, kt * 128:(kt + 1) * 128],
                                    rhs=qT[:, qs:qs + QB],
                                )
                            p = pxp.tile([128, 2, QB], BF16, tag="p")
                            nc.scalar.activation(p, sp, Act.Exp)
                            for u in range(2):
                                pieces.append((p, u, 0, 2 * j + u))
                        # diagonal key tiles: restrict to valid queries
                        for dp in range(2):
                            s0 = 2 * dp * 128
                            sp = pairp.tile([128, 2, QB], FP32, tag="sp")
                            for u in range(2):
                                kt = 4 * qb + 2 * dp + u
                                # compute from the pair start s0 (not the
                                # tile's own diagonal s) so the paired exp
                                # below reads only freshly-written PSUM
                                nc.tensor.matmul(
                                    sp[:, u, s0:QB],
                                    lhsT=kT[:, kt * 128:(kt + 1) * 128],
                                    rhs=qT[:, qs + s0:qs + QB],
                                )
                            p = pxp.tile([128, 2, QB], BF16, tag="p")
                            nc.scalar.activation(p[:, :, s0:QB], sp[:, :, s0:QB], Act.Exp)
                            for u in range(2):
                                dq = 2 * dp + u
                                s = dq * 128
                                # causal edge mask on DVE: it must NOT share a
                                # queue with the broadcast, which waits on the
                                # slow reciprocal (the in-order queue would
                                # make the next head's AV inherit that wait)
                                nc.vector.tensor_mul(
                                    p[:, u, s:s + 128], p[:, u, s:s + 128], mask_sb
                                )
                                pieces.append((p, u, s, 4 * qb + dq))
                        return pieces

                    def attn_av(h, qb, pieces, ytc):
                        """AV accumulation + softmax normalization for a head
                        whose scores were emitted one pipeline slot earlier."""
                        hp, off = h // 2, (h % 2) * 64
                        last = pieces[-1][3]
                        yacc = accp.tile([128, 512], FP32, tag="acc")
                        for p, u, s, kt in pieces:
                            nc.tensor.matmul(
                                yacc[0:65, s:QB],
                                lhsT=v_sb[:, h, kt, :], rhs=p[:, u, s:QB],
                                start=(kt == 0), stop=(kt == last),
                            )
                        # evacuate [y_un; rowsum] to SBUF on ACT right away:
                        # this releases the PSUM bank without waiting for the
                        # slow DVE reciprocal, which otherwise paces the whole
                        # acc-pool rotation
                        yun = yup.tile([65, QB], FP32, tag="yun")
                        nc.scalar.activation(yun, yacc[0:65, :], Act.Copy)
                        # normalize: y^T[d, q] * (1/rowsum[q])
                        rrec = rrp.tile([1, QB], FP32, tag="rrec")
                        nc.vector.reciprocal(rrec, yun[64:65, :])
                        rbs = rbp.tile([64, QB], FP32, tag="rbs")
                        nc.gpsimd.partition_broadcast(rbs, rrec)
                        nc.vector.tensor_mul(ytc[off:off + 64, hp, :], yun[0:64, :], rbs)

                    def out_proj(qb):
                        """Out-projection of block qb (its AllGather was
                        launched one block ago); copies ride ACT."""
                        yf = yfp.tile([128, 8, QB], BF16, tag="yf")
                        for a in range(8):
                            nc.sync.dma_start(
                                out=yf[:, a, :],
                                in_=yfull[qb][a * 128:(a + 1) * 128, :],
                            )
                        for tl in range(4):
                            po = accp.tile([128, 512], FP32, tag="acc")
                            for a in range(8):
                                nc.tensor.matmul(
                                    po,
                                    lhsT=yf[:, a, tl * 128:(tl + 1) * 128],
                                    rhs=wout_sb[:, a, :],
                                    start=(a == 0), stop=(skip_bias and a == 7),
                                )
                            if not skip_bias:
                                nc.tensor.matmul(
                                    po,
                                    lhsT=onesb_sb[0:1, :],
                                    rhs=bfull_sb[0:1, :],
                                    start=False, stop=True,
                                )
                            ob = obp.tile([128, 512], FP32, tag="ob")
                            nc.scalar.activation(ob, po, Act.Copy)
                            nc.sync.dma_start(
                                out=out[qb * QB + tl * 128:qb * QB + (tl + 1) * 128, :],
                                in_=ob,
                            )

                    for ch in range(NQB):
                        qkv_chunk(ch)
                        qb = ch
                        # out-projection of the previous block BEFORE this
                        # block's attention: its DMAs must not queue behind
                        # this block's (late-firing) ytc DMA
                        if qb >= 1:
                            out_proj(qb - 1)
                        ytc = ytp.tile([128, HPT, QB], BF16, tag="ytc")
                        # software-pipeline across heads: head h's scores are
                        # emitted before head h-1's AV, so the tensor engine
                        # always has score matmuls queued while ACT runs exp
                        prev = None
                        for h in range(HPC):
                            # scores of head h before AV of head h-1: the
                            # masks land on DVE ahead of the slow reciprocal,
                            # and the AV's exp dependency resolved a slot ago
                            pieces = attn_scores(h, qb)
                            if prev is not None:
                                attn_av(h - 1, qb, prev, ytc)
                            prev = pieces
                        attn_av(HPC - 1, qb, prev, ytc)
                        nc.sync.dma_start(
                            out=yloc[qb].rearrange("(hp p) t -> p hp t", p=128),
                            in_=ytc,
                        )
                        nc.gpsimd.collective_compute(
                            "AllGather",
                            mybir.AluOpType.bypass,
                            replica_groups=groups,
                            ins=[yloc[qb].opt()],
                            outs=[yfull[qb].opt()],
                        )
                    out_proj(NQB - 1)
    nc.compile()
    return nc


def shard_inputs(x, W_qkv, b_qkv, W_out, b_out):
    """Build the 8 per-core input maps."""
    x = np.asarray(x, dtype=np.float32)
    W_qkv = np.asarray(W_qkv, dtype=np.float32)
    b_qkv = np.asarray(b_qkv, dtype=np.float32)
    W_out = np.asarray(W_out, dtype=np.float32)
    b_out = np.asarray(b_out, dtype=np.float32)

    # p[i, j] valid iff query j >= key i within the diagonal 128x128 tile
    mask = (np.arange(128)[None, :] >= np.arange(128)[:, None]).astype(
        ml_dtypes.bfloat16
    )
    onesb = np.ones((1, 128), dtype=ml_dtypes.bfloat16)

    in_maps = []
    for c in range(NCORES):
        b = c // 2
        hh = (c % 2) * HPC  # first head on this core
        col = hh * DH       # 512-wide column slice per kind
        oc = (c % 2) * 512  # out-projection column half for this core
        wout_bf = np.ascontiguousarray(
            W_out[:, oc:oc + 512].astype(ml_dtypes.bfloat16)
        )
        bfull = np.ascontiguousarray(
            b_out[None, oc:oc + 512].astype(ml_dtypes.bfloat16)
        )
        xT = np.ascontiguousarray(x[b].T.astype(ml_dtypes.bfloat16))
        wq = W_qkv[:, 0 * C + col:0 * C + col + 512] * (1.0 / 8.0)
        wk = W_qkv[:, 1 * C + col:1 * C + col + 512]
        wv = W_qkv[:, 2 * C + col:2 * C + col + 512]
        wqkv_c = np.ascontiguousarray(
            np.concatenate([wq, wk, wv], axis=1).astype(ml_dtypes.bfloat16)
        )
        bq = b_qkv[0 * C + col:0 * C + col + 512] * (1.0 / 8.0)
        bk = b_qkv[1 * C + col:1 * C + col + 512]
        bqk_c = np.stack(
            [bq[hp * 128:(hp + 1) * 128] for hp in range(4)]
            + [bk[hp * 128:(hp + 1) * 128] for hp in range(4)],
            axis=1,
        ).astype(np.float32)
        bv_c = np.ascontiguousarray(
            b_qkv[2 * C + col:2 * C + col + 512][None, :].astype(ml_dtypes.bfloat16)
        )
        in_maps.append(
            {
                "xT": xT,
                "wqkv": wqkv_c,
                "wout": wout_bf,
                "bqk": np.ascontiguousarray(bqk_c),
                "bv": bv_c,
                "bfull": bfull,
                "mask": mask,
                "onesb": onesb,
            }
        )
    return in_maps


def gather_outputs(results):
    full = np.zeros((B, T, C), dtype=np.float32)
    for c, r in enumerate(results):
        o = np.asarray(r["out"])  # [T, 512]: this core's output column half
        b, half = c // 2, c % 2
        full[b, :, half * 512:(half + 1) * 512] = o
    return full


_CACHED = {}


def kernel(x, W_qkv, b_qkv, W_out, b_out):
    from concourse.bass_utils import run_bass_kernel_spmd

    zb = bool(
        np.all(np.asarray(b_qkv) == 0) and np.all(np.asarray(b_out) == 0)
    )
    key = f"nc{zb}"
    if key not in _CACHED:
        _CACHED[key] = build_program(skip_bias=zb)
    nc = _CACHED[key]
    in_maps = shard_inputs(x, W_qkv, b_qkv, W_out, b_out)
    res = run_bass_kernel_spmd(nc, in_maps, list(range(NCORES)))
    return gather_outputs(res.results)


if __name__ == "__main__":
    import reference

    inputs = reference.setup_inputs()
    expected = np.asarray(reference.reference(**inputs))
    actual = kernel(**{k: np.asarray(v) for k, v in inputs.items()})
    err = np.linalg.norm(actual - expected) / np.linalg.norm(expected)
    print("Relative error:", err)


# revision 23
# speedup vs baseline: 1.1245x; 1.1245x over previous
"""Causal self-attention (B=4, T=2048, C=1024, H=16) on 8 TRN2 NeuronCores.

Sharding: core c handles batch element c//2 with heads (c%2)*8 .. +8
(tensor-parallel over heads, paired per batch element).

Single merged SPMD pipeline per 512-token chunk ch:
  1. QKV projection for chunk ch: Q^T,K^T accumulated as [dh,t] pairs in
     PSUM, copied to SBUF bf16 on the Pool engine; V in [t,dh] layout with
     an appended ones-column (rowsum rides the AV matmul for free).
  2. Attention for query block qb=ch (needs K/V only for chunks <= ch):
     scores S^T[k,q] = K-tile^T . Q^T restricted to the exact 128-granular
     causal triangle, computed in PSUM *pairs* so one ACT exp instruction
     covers [128, 2, 512] (amortizes ACT per-instruction overhead), causal
     edge handled by a [128,128] tril mask multiply on DVE, then AV with
     the ones-augmented V accumulates [y^T; rowsum] in PSUM.  Rowsum
     reciprocal via the fast DVE approximation, partition-broadcast on the
     Pool engine, normalize on DVE.
  3. Pairwise AllToAll exchanges token-halves of y^T (each core of a pair
     keeps 256 of each 512 query block with all 1024 features), then the
     out-projection runs only on this core's half of the tokens.
     Out-projection for block qb is emitted during attention qb+1 so the
     collective latency hides under compute.

Host side only shards/relays inputs and re-interleaves the 8 output shards.
"""

import math
import sys

import numpy as np

for _p in ("/opt/trn_rl_repo",):
    if _p not in sys.path:
        sys.path.insert(0, _p)

import ml_dtypes  # noqa: E402

import concourse.bass as bass  # noqa: E402
import concourse.bacc as bacc  # noqa: E402
import concourse.mybir as mybir  # noqa: E402
import concourse.tile as tile  # noqa: E402

FP32 = mybir.dt.float32
BF16 = mybir.dt.bfloat16
Act = mybir.ActivationFunctionType

B, T, C, H, DH = 4, 2048, 1024, 16, 64
NCORES = 8
HPC = 8        # heads per core
HPT = HPC // 2  # head-pair tiles of 128 partitions
QB = 512       # query block / token chunk
NQB = T // QB


def build_program(skip_bias=False):
    nc = bacc.Bacc(None, num_devices=NCORES)
    xT = nc.declare_dram_parameter("xT", [C, T], BF16, isOutput=False)
    wqkv = nc.declare_dram_parameter("wqkv", [C, 3 * 512], BF16, isOutput=False)
    # per-core 512-column slice of W_out (out-projection is column-split)
    wout = nc.declare_dram_parameter("wout", [C, 512], BF16, isOutput=False)
    bqk = nc.declare_dram_parameter("bqk", [128, 8], FP32, isOutput=False)
    bv = nc.declare_dram_parameter("bv", [1, 512], BF16, isOutput=False)
    bfull = nc.declare_dram_parameter("bfull", [1, 512], BF16, isOutput=False)
    maskp = nc.declare_dram_parameter("mask", [128, 128], BF16, isOutput=False)
    onesbp = nc.declare_dram_parameter("onesb", [1, 128], BF16, isOutput=False)
    out = nc.declare_dram_parameter("out", [T, 512], FP32, isOutput=True)

    groups = [[0, 1], [2, 3], [4, 5], [6, 7]]

    with tile.TileContext(nc, num_cores=NCORES) as tc:
        with (
            tc.tile_pool(name="const", bufs=1) as cpool,
            tc.tile_pool(name="dram", bufs=1, space="DRAM") as dpool,
        ):
            # ---- constants ----
            wout_sb = cpool.tile([128, 8, 512], BF16)
            for a in range(8):
                nc.sync.dma_start(out=wout_sb[:, a, :], in_=wout[a * 128:(a + 1) * 128, :])
            mask_sb = cpool.tile([128, 128], BF16)
            nc.sync.dma_start(out=mask_sb, in_=maskp[:, :])
            onesb_sb = cpool.tile([1, 128], BF16)
            nc.sync.dma_start(out=onesb_sb, in_=onesbp[:, :])
            bqk_sb = cpool.tile([128, 8], FP32)
            nc.sync.dma_start(out=bqk_sb, in_=bqk[:, :])
            bv_sb = cpool.tile([1, 512], BF16)
            nc.sync.dma_start(out=bv_sb, in_=bv[:, :])
            bfull_sb = cpool.tile([1, 512], BF16)
            nc.sync.dma_start(out=bfull_sb, in_=bfull[:, :])

            # exchange buffers (dram): per query block, local y^T half and
            # the pairwise-allgathered full y^T
            yloc = [
                dpool.tile([512, QB], BF16, tag=f"yloc{i}", name=f"yloc{i}")
                for i in range(NQB)
            ]
            yfull = [
                dpool.tile([1024, QB], BF16, tag=f"yfull{i}", name=f"yfull{i}")
                for i in range(NQB)
            ]

            with tc.tile_pool(name="persist", bufs=1) as ppool:
                # ---- persistent activations ----
                # qk=0 -> Q^T, qk=1 -> K^T, laid out [dh, qk, hp, t]
                qkT_sb = ppool.tile([128, 2, HPT, T], BF16, name="qkT")
                v_sb = ppool.tile([128, HPC, T // 128, 65], BF16, name="v")
                nc.vector.memset(v_sb[:, :, :, 64], 1.0)

                with (
                    tc.tile_pool(name="wq", bufs=1) as wpool,
                    tc.tile_pool(name="xch", bufs=2) as xpool,
                    tc.tile_pool(name="pair", bufs=2, space="PSUM") as pairp,
                    tc.tile_pool(name="accp", bufs=4, space="PSUM") as accp,
                    tc.tile_pool(name="pexp", bufs=16) as pxp,
                    tc.tile_pool(name="rr", bufs=2) as rrp,
                    tc.tile_pool(name="yun", bufs=4) as yup,
                    tc.tile_pool(name="rb", bufs=2) as rbp,
                    tc.tile_pool(name="ytc", bufs=2) as ytp,
                    tc.tile_pool(name="yf", bufs=2) as yfp,
                    tc.tile_pool(name="ob", bufs=2) as obp,
                ):
                    w_sb = wpool.tile([128, 8, 3 * 512], BF16)
                    for a in range(8):
                        nc.sync.dma_start(out=w_sb[:, a, :], in_=wqkv[a * 128:(a + 1) * 128, :])

                    def qkv_units(ch):
                        """Emit the x-chunk DMAs now; return closures (4 Q/K
                        pair-groups + 4 V groups) interleaved into the
                        preceding attention block.  All PSUM->SBUF copies ride
                        ACT, whose queue never waits on the reciprocal."""
                        ts = ch * QB
                        xt = xpool.tile([128, 8, QB], BF16, tag="xt")
                        for a in range(8):
                            nc.sync.dma_start(out=xt[:, a, :], in_=xT[a * 128:(a + 1) * 128, ts:ts + QB])

                        def qk_unit(hp):
                            pt = pairp.tile([128, 2, QB], FP32, tag="sp")
                            for kind in range(2):  # 0=q, 1=k
                                wcol = kind * 512 + hp * 128
                                for a in range(8):
                                    nc.tensor.matmul(
                                        pt[:, kind, :],
                                        lhsT=w_sb[:, a, wcol:wcol + 128],
                                        rhs=xt[:, a, :],
                                        start=(a == 0), stop=(a == 7),
                                    )
                            dst = qkT_sb[:, :, hp, ts:ts + QB]
                            if skip_bias:
                                nc.scalar.activation(dst, pt, Act.Copy)
                            else:
                                for kind in range(2):
                                    nc.scalar.activation(
                                        dst[:, kind, :], pt[:, kind, :],
                                        Act.Identity,
                                        bias=bqk_sb[:, kind * 4 + hp:kind * 4 + hp + 1],
                                    )

                        def v_unit(tl):
                            accv = accp.tile([128, 512], FP32, tag="acc")
                            for a in range(8):
                                nc.tensor.matmul(
                                    accv,
                                    lhsT=xt[:, a, tl * 128:(tl + 1) * 128],
                                    rhs=w_sb[:, a, 1024:1536],
                                    start=(a == 0), stop=(skip_bias and a == 7),
                                )
                            if not skip_bias:
                                nc.tensor.matmul(
                                    accv,
                                    lhsT=onesb_sb[0:1, :],
                                    rhs=bv_sb[0:1, :],
                                    start=False, stop=True,
                                )
                            tt = ch * 4 + tl
                            nc.scalar.activation(
                                v_sb[:, :, tt, 0:64],
                                accv.rearrange("p (h d) -> p h d", d=64),
                                Act.Copy,
                            )

                        qks = [lambda hp=hp: qk_unit(hp) for hp in range(HPT)]
                        vs = [lambda tl=tl: v_unit(tl) for tl in range(4)]
                        return qks, vs

                    def attn_scores(h, qb):
                        """Scores + exp + causal mask for one head; returns
                        the P-tile pieces the (later) AV stage consumes."""
                        hp, off = h // 2, (h % 2) * 64
                        qs = qb * QB
                        qT = qkT_sb[off:off + 64, 0, hp, :]
                        kT = qkT_sb[off:off + 64, 1, hp, :]
                        pieces = []
                        # off-diagonal key tiles, two per PSUM pair
                        for j in range(2 * qb):
                            sp = pairp.tile([128, 2, QB], FP32, tag="sp")
                            for u in range(2):
                                kt = 2 * j + u
                                nc.tensor.matmul(
                                    sp[:, u, :],
                                    lhsT=kT[:, kt * 128:(kt + 1) * 128],
                                    rhs=qT[:, qs:qs + QB],
                                )
                            p = pxp.tile([128, 2, QB], BF16, tag="p")
                            nc.scalar.activation(p, sp, Act.Exp)
                            for u in range(2):
                                pieces.append((p, u, 0, 2 * j + u))
                        # diagonal key tiles: restrict to valid queries
                        for dp in range(2):
                            s0 = 2 * dp * 128
                            sp = pairp.tile([128, 2, QB], FP32, tag="sp")
                            for u in range(2):
                                kt = 4 * qb + 2 * dp + u
                                # compute from the pair start s0 (not the
                                # tile's own diagonal s) so the paired exp
                                # below reads only freshly-written PSUM
                                nc.tensor.matmul(
                                    sp[:, u, s0:QB],
                                    lhsT=kT[:, kt * 128:(kt + 1) * 128],
                                    rhs=qT[:, qs + s0:qs + QB],
                                )
                            p = pxp.tile([128, 2, QB], BF16, tag="p")
                            nc.scalar.activation(p[:, :, s0:QB], sp[:, :, s0:QB], Act.Exp)
                            for u in range(2):
                                dq = 2 * dp + u
                                s = dq * 128
                                # causal edge mask on DVE: it must NOT share a
                                # queue with the broadcast, which waits on the
                                # slow reciprocal (the in-order queue would
                                # make the next head's AV inherit that wait)
                                nc.vector.tensor_mul(
                                    p[:, u, s:s + 128], p[:, u, s:s + 128], mask_sb
                                )
                                pieces.append((p, u, s, 4 * qb + dq))
                        return pieces

                    def attn_av(h, qb, pieces, ytc):
                        """AV accumulation + softmax normalization for a head
                        whose scores were emitted one pipeline slot earlier."""
                        hp, off = h // 2, (h % 2) * 64
                        last = pieces[-1][3]
                        yacc = accp.tile([128, 512], FP32, tag="acc")
                        for p, u, s, kt in pieces:
                            nc.tensor.matmul(
                                yacc[0:65, s:QB],
                                lhsT=v_sb[:, h, kt, :], rhs=p[:, u, s:QB],
                                start=(kt == 0), stop=(kt == last),
                            )
                        # evacuate [y_un; rowsum] to SBUF on ACT right away:
                        # this releases the PSUM bank without waiting for the
                        # slow DVE reciprocal, which otherwise paces the whole
                        # acc-pool rotation
                        yun = yup.tile([65, QB], FP32, tag="yun")
                        nc.vector.tensor_copy(yun, yacc[0:65, :])
                        # normalize: y^T[d, q] * (1/rowsum[q])
                        rrec = rrp.tile([1, QB], FP32, tag="rrec")
                        nc.vector.reciprocal(rrec, yun[64:65, :])
                        rbs = rbp.tile([64, QB], FP32, tag="rbs")
                        nc.gpsimd.partition_broadcast(rbs, rrec)
                        nc.vector.tensor_mul(ytc[off:off + 64, hp, :], yun[0:64, :], rbs)

                    def op_units(qb):
                        """Out-projection of block qb as 4 closures; emitted
                        late in the following attention block so its
                        AllGather has completed."""
                        yf = yfp.tile([128, 8, QB], BF16, tag="yf")

                        def tl_unit(tl):
                            if tl == 0:
                                for a in range(8):
                                    nc.sync.dma_start(
                                        out=yf[:, a, :],
                                        in_=yfull[qb][a * 128:(a + 1) * 128, :],
                                    )
                            po = accp.tile([128, 512], FP32, tag="acc")
                            for a in range(8):
                                nc.tensor.matmul(
                                    po,
                                    lhsT=yf[:, a, tl * 128:(tl + 1) * 128],
                                    rhs=wout_sb[:, a, :],
                                    start=(a == 0), stop=(skip_bias and a == 7),
                                )
                            if not skip_bias:
                                nc.tensor.matmul(
                                    po,
                                    lhsT=onesb_sb[0:1, :],
                                    rhs=bfull_sb[0:1, :],
                                    start=False, stop=True,
                                )
                            ob = obp.tile([128, 512], FP32, tag="ob")
                            nc.scalar.activation(ob, po, Act.Copy)
                            nc.sync.dma_start(
                                out=out[qb * QB + tl * 128:qb * QB + (tl + 1) * 128, :],
                                in_=ob,
                            )

                        return [lambda tl=tl: tl_unit(tl) for tl in range(4)]

                    # warm-up: QKV for chunk 0 runs undisturbed
                    qks0, vs0 = qkv_units(0)
                    for unit in qks0 + vs0:
                        unit()
                    for qb in range(NQB):
                        # filler schedule: Q/K and V groups of the next chunk
                        # early in the block, out-projection of the previous
                        # block late (after its AllGather has landed)
                        slots = [[] for _ in range(HPC)]
                        if qb + 1 < NQB:
                            qks, vs = qkv_units(qb + 1)
                            for i in range(4):
                                slots[i].append(qks[i])
                                slots[i + 1].append(vs[i])
                        if qb >= 1:
                            for i, unit in enumerate(op_units(qb - 1)):
                                slots[4 + i].append(unit)
                        ytc = ytp.tile([128, HPT, QB], BF16, tag="ytc")
                        prev = None
                        for h in range(HPC):
                            # scores of head h before AV of head h-1: the
                            # masks land on DVE ahead of the slow reciprocal,
                            # and the AV's exp dependency resolved a slot ago
                            pieces = attn_scores(h, qb)
                            if prev is not None:
                                attn_av(h - 1, qb, prev, ytc)
                            prev = pieces
                            for unit in slots[h]:
                                unit()
                        attn_av(HPC - 1, qb, prev, ytc)
                        nc.sync.dma_start(
                            out=yloc[qb].rearrange("(hp p) t -> p hp t", p=128),
                            in_=ytc,
                        )
                        nc.gpsimd.collective_compute(
                            "AllGather",
                            mybir.AluOpType.bypass,
                            replica_groups=groups,
                            ins=[yloc[qb].opt()],
                            outs=[yfull[qb].opt()],
                        )
                    for unit in op_units(NQB - 1):
                        unit()
    nc.compile()
    return nc


def shard_inputs(x, W_qkv, b_qkv, W_out, b_out):
    """Build the 8 per-core input maps."""
    x = np.asarray(x, dtype=np.float32)
    W_qkv = np.asarray(W_qkv, dtype=np.float32)
    b_qkv = np.asarray(b_qkv, dtype=np.float32)
    W_out = np.asarray(W_out, dtype=np.float32)
    b_out = np.asarray(b_out, dtype=np.float32)

    # p[i, j] valid iff query j >= key i within the diagonal 128x128 tile
    mask = (np.arange(128)[None, :] >= np.arange(128)[:, None]).astype(
        ml_dtypes.bfloat16
    )
    onesb = np.ones((1, 128), dtype=ml_dtypes.bfloat16)

    in_maps = []
    for c in range(NCORES):
        b = c // 2
        hh = (c % 2) * HPC  # first head on this core
        col = hh * DH       # 512-wide column slice per kind
        oc = (c % 2) * 512  # out-projection column half for this core
        wout_bf = np.ascontiguousarray(
            W_out[:, oc:oc + 512].astype(ml_dtypes.bfloat16)
        )
        bfull = np.ascontiguousarray(
            b_out[None, oc:oc + 512].astype(ml_dtypes.bfloat16)
        )
        xT = np.ascontiguousarray(x[b].T.astype(ml_dtypes.bfloat16))
        wq = W_qkv[:, 0 * C + col:0 * C + col + 512] * (1.0 / 8.0)
        wk = W_qkv[:, 1 * C + col:1 * C + col + 512]
        wv = W_qkv[:, 2 * C + col:2 * C + col + 512]
        wqkv_c = np.ascontiguousarray(
            np.concatenate([wq, wk, wv], axis=1).astype(ml_dtypes.bfloat16)
        )
        bq = b_qkv[0 * C + col:0 * C + col + 512] * (1.0 / 8.0)
        bk = b_qkv[1 * C + col:1 * C + col + 512]
        bqk_c = np.stack(
            [bq[hp * 128:(hp + 1) * 128] for hp in range(4)]
            + [bk[hp * 128:(hp + 1) * 128] for hp in range(4)],
            axis=1,
        ).astype(np.float32)
        bv_c = np.ascontiguousarray(
            b_qkv[2 * C + col:2 * C + col + 512][None, :].astype(ml_dtypes.bfloat16)
        )
        in_maps.append(
            {
                "xT": xT,
                "wqkv": wqkv_c,
                "wout": wout_bf,
                "bqk": np.ascontiguousarray(bqk_c),
                "bv": bv_c,
                "bfull": bfull,
                "mask": mask,
                "onesb": onesb,
            }
        )
    return in_maps


def gather_outputs(results):
    full = np.zeros((B, T, C), dtype=np.float32)
    for c, r in enumerate(results):
        o = np.asarray(r["out"])  # [T, 512]: this core's output column half
        b, half = c // 2, c % 2
        full[b, :, half * 512:(half + 1) * 512] = o
    return full


_CACHED = {}


def kernel(x, W_qkv, b_qkv, W_out, b_out):
    from concourse.bass_utils import run_bass_kernel_spmd

    zb = bool(
        np.all(np.asarray(b_qkv) == 0) and np.all(np.asarray(b_out) == 0)
    )
    key = f"nc{zb}"
    if key not in _CACHED:
        _CACHED[key] = build_program(skip_bias=zb)
    nc = _CACHED[key]
    in_maps = shard_inputs(x, W_qkv, b_qkv, W_out, b_out)
    res = run_bass_kernel_spmd(nc, in_maps, list(range(NCORES)))
    return gather_outputs(res.results)


if __name__ == "__main__":
    import reference

    inputs = reference.setup_inputs()
    expected = np.asarray(reference.reference(**inputs))
    actual = kernel(**{k: np.asarray(v) for k, v in inputs.items()})
    err = np.linalg.norm(actual - expected) / np.linalg.norm(expected)
    print("Relative error:", err)
